# revision 21
# baseline (speedup 1.0000x reference)
"""Trainium2 Bass kernel for nn_DialatedRNN (D=4, I=H=O=2048, batch=1).

Strategy (tensor-parallel over 8 NeuronCores):
  - Each core owns a 256-wide slice of H for every gate (r/z/n) of every
    active GRU layer, plus a 256-wide column slice of the output Linear.
  - Gate matvecs run on the TensorEngine with the (tiny) x / h vectors as
    the *stationary* operand and the fp16-packed transposed weights as the
    *moving* operand, accumulating over 16 K-tiles into PSUM.  Biases are
    folded in with one extra K=1 matmul per gate group.
  - The elementwise GRU tail (sigmoid/tanh/blend) runs on DVE+ACT on
    partition 0; the composite vector is accumulated *in column layout*
    directly in PSUM via K=1/M=128 matmuls so the final out_w matvec needs
    no transpose.
  - Each core returns one packed [1, 4096] f32 buffer:
    [ partial_out(2048) | new_mem(4x256) | new_res(4x256) ].
    The host sums the 8 partial outputs, adds out_b, and reassembles the
    H-sharded new_res / new_mem.

All weights are streamed as fp16 (f32 accumulate in PSUM): ~26 MB per core,
which is the memory-roofline-relevant traffic.  End-to-end rel-err vs the
f32 reference is ~3e-4.
"""

import sys
import types

import numpy as np

import concourse.bacc as bacc
import concourse.mybir as mybir
import concourse.tile as tile
from concourse.bass_utils import run_bass_kernel_spmd


def _ensure_ntff_hook():
    """The agent image's ``antenv`` package lacks ``axon_hooks``, which
    ``run_bass_kernel_spmd(trace=True)`` imports under axon.  Provide the
    tiny get/set module and register the same ctypes-based NTFF hook the
    axon boot would have installed, so profiling works."""
    try:
        from antenv.axon_hooks import get_axon_ntff_profile_hook  # noqa: F401
        return
    except ImportError:
        pass
    mod = types.ModuleType("antenv.axon_hooks")
    holder = {"hook": None}
    mod.set_axon_ntff_profile_hook = lambda h: holder.__setitem__("hook", h)
    mod.get_axon_ntff_profile_hook = lambda: holder["hook"]
    try:
        import antenv
        sys.modules["antenv.axon_hooks"] = mod
        antenv.axon_hooks = mod
        from trn_agent_boot.trn_boot import _ntff_profile_via_ctypes
        hook = _ntff_profile_via_ctypes("/opt/axon/libaxon_pjrt.so")
        if hook is not None:
            mod.set_axon_ntff_profile_hook(hook)
    except Exception:
        pass


_ensure_ntff_hook()

F16 = np.float16
F32 = np.float32

D, I, H, O = 4, 2048, 2048, 2048
NCORES = 8
CH = H // NCORES            # 256: per-core slice of H (and of O columns)
KT = I // 128               # 16 K-tiles for the gate contractions
F_RZ = KT * 2 * CH          # 8192 fp16 elems/partition for a packed rz block
F_NN = KT * CH              # 4096 for a packed n block
F_LAYER = 2 * F_RZ + 2 * F_NN   # 24576
F_OW = 2 * O                # 4096: out_w column-slice, 2 K-tiles x 2048

AUX_MEM = 0                 # [D*CH]   f32 memory rows (per-core slice)
AUX_RES = D * CH            # [D*CH]   f32 residual rows
AUX_CV = 2 * D * CH         # [3*D]    composite scales (fused | cpos | cres)
AUX_BASE = AUX_CV + 3 * D   # [CH]     base composite (inactive layers)
AUX_LEN = AUX_BASE + CH

OUT_P = 0                   # packed output layout
OUT_MEM = O
OUT_RES = O + D * CH
OUT_LEN = O + 2 * D * CH    # 4096

_KERNEL_CACHE: dict = {}
_PACK_CACHE: dict = {}


def _active_layers(step: int):
    return tuple(i for i in range(D) if step % ((i + 1) ** 2) == 0)


def _build_nc(active):
    """Build + compile the per-core SPMD Bass program for a given active set."""
    na = len(active)
    has_inactive = na < D
    # per active layer: does the positional weight equal the residual weight?
    fuse = tuple(1.0 / (a + 1) ** 2 == 1.0 / (i + 1) ** 2
                 for a, i in enumerate(active))

    nc = bacc.Bacc("TRN2", target_bir_lowering=False, debug=False)
    dt = mybir.dt

    wpack = nc.dram_tensor("wpack", [128, na * F_LAYER + F_OW], dt.float16,
                           kind="ExternalInput").ap()
    xt = nc.dram_tensor("xt", [128, KT], dt.float16, kind="ExternalInput").ap()
    ht = nc.dram_tensor("ht", [128, na * KT], dt.float16,
                        kind="ExternalInput").ap()
    bias16 = nc.dram_tensor("bias16", [1, na * 1024], dt.float16,
                            kind="ExternalInput").ap()
    aux32 = nc.dram_tensor("aux32", [1, AUX_LEN], dt.float32,
                           kind="ExternalInput").ap()
    outbuf = nc.dram_tensor("outbuf", [1, OUT_LEN], dt.float32,
                            kind="ExternalOutput").ap()

    with tile.TileContext(nc) as tc:
        with (
            tc.tile_pool(name="wp", bufs=4) as wp,
            tc.tile_pool(name="sp", bufs=1) as sp,
            tc.tile_pool(name="wk", bufs=2) as wk,
            tc.tile_pool(name="pp", bufs=2, space="PSUM") as pp,
        ):
            # Small inputs ride SWDGE (gpsimd) so the HWDGE rings start
            # streaming weights immediately instead of paying 4 serialized
            # ~2.6us fixed costs first.
            xt_sb = sp.tile([128, KT], dt.float16)
            nc.gpsimd.dma_start(out=xt_sb[:, :], in_=xt[:, :])
            ht_sb = sp.tile([128, na * KT], dt.float16)
            nc.gpsimd.dma_start(out=ht_sb[:, :], in_=ht[:, :])
            bias_sb = sp.tile([1, na * 1024], dt.float16)
            nc.gpsimd.dma_start(out=bias_sb[:, :], in_=bias16[:, :])
            aux_sb = sp.tile([1, AUX_LEN], dt.float32)
            nc.gpsimd.dma_start(out=aux_sb[:, :], in_=aux32[:, :])

            ones16 = sp.tile([1, 1], dt.float16)
            nc.vector.memset(ones16[:, :], 1.0)
            one32 = sp.tile([1, 1], dt.float32)
            nc.vector.memset(one32[:, :], 1.0)

            staging = sp.tile([1, OUT_LEN], dt.float32)
            if has_inactive:
                nc.vector.memset(staging[:, :], 0.0)

            # out_w slice rides SWDGE early: keeps its 2MB off the HWDGE
            # gate-weight stream (whose end time bounds the serial tail) and
            # it is resident long before the final matvec needs it.
            ow_sb = []
            for t in range(2):
                t_ow = wp.tile([128, O], dt.float16, tag="ow", bufs=2,
                               name=f"ow_{t}")
                nc.gpsimd.dma_start(
                    out=t_ow[:, :],
                    in_=wpack[:, na * F_LAYER + t * O:na * F_LAYER + (t + 1) * O])
                ow_sb.append(t_ow)

            psum_tr = pp.tile([128, 2], dt.float32, tag="tr", bufs=1)
            # HAM keep-warm bank: per-chunk dummy matmuls accumulate here
            psum_du = pp.tile([1, 512], dt.float32, tag="du", bufs=1)
            n_contrib = (1 if has_inactive else 0) + 2 * na
            contrib = [0]  # mutable counter shared across emitters

            # Round-robin weight DMAs over both HWDGE rings (SP + ACT).
            dma_rr = [0]

            def wdma(out_ap, off, flen):
                eng = nc.sync if dma_rr[0] % 2 == 0 else nc.scalar
                dma_rr[0] += 1
                eng.dma_start(out=out_ap, in_=wpack[:, off:off + flen])

            def tr_mm(row_ap, scale_ap):
                """psum_tr[:, t] += row[t*128:(t+1)*128].T * scale, t=0,1.

                start=True clears the PSUM tile's whole bank region, so only
                the very first matmul may carry it; later columns accumulate
                onto the cleared zeros.
                """
                for t in range(2):
                    nc.tensor.matmul(
                        out=psum_tr[:, t:t + 1],
                        lhsT=row_ap[:, t * 128:(t + 1) * 128],
                        rhs=scale_ap,
                        start=(contrib[0] == 0 and t == 0),
                        stop=(contrib[0] == n_contrib - 1 and t == 1),
                        skip_group_check=True,
                    )
                contrib[0] += 1

            if has_inactive:
                tr_mm(aux_sb[:, AUX_BASE:AUX_BASE + CH], one32[:, :])

            for idx, i in enumerate(active):
                base = idx * F_LAYER
                rzih = []
                rzhh = []
                nnih = []
                nnhh = []
                # 512KB chunks, interleaved in exactly the order the PE
                # consumes them: rz quarters (4 k-tiles each), nn halves.
                for q in range(4):
                    t_rzih = wp.tile([128, F_RZ // 4], dt.float16, tag="rzih",
                                     bufs=8, name=f"rzih_{idx}_{q}")
                    wdma(t_rzih[:, :], base + q * F_RZ // 4, F_RZ // 4)
                    rzih.append(t_rzih)
                    t_rzhh = wp.tile([128, F_RZ // 4], dt.float16, tag="rzhh",
                                     bufs=8, name=f"rzhh_{idx}_{q}")
                    wdma(t_rzhh[:, :], base + F_RZ + q * F_RZ // 4, F_RZ // 4)
                    rzhh.append(t_rzhh)
                for h in range(2):
                    t_nnih = wp.tile([128, F_NN // 2], dt.float16, tag="nnih",
                                     bufs=4, name=f"nnih_{idx}_{h}")
                    wdma(t_nnih[:, :], base + 2 * F_RZ + h * F_NN // 2,
                         F_NN // 2)
                    nnih.append(t_nnih)
                    t_nnhh = wp.tile([128, F_NN // 2], dt.float16, tag="nnhh",
                                     bufs=4, name=f"nnhh_{idx}_{h}")
                    wdma(t_nnhh[:, :], base + 2 * F_RZ + F_NN + h * F_NN // 2,
                         F_NN // 2)
                    nnhh.append(t_nnhh)

                psum_rz = pp.tile([1, 2 * CH], dt.float32, tag="rz",
                                  name=f"psum_rz_{idx}")
                psum_nn = pp.tile([1, 2 * CH], dt.float32, tag="nn",
                                  name=f"psum_nn_{idx}")

                for k in range(KT):
                    hk, ko = divmod(k, 4)
                    nc.tensor.matmul(
                        out=psum_rz[:, :],
                        lhsT=xt_sb[:, k:k + 1],
                        rhs=rzih[hk][:, ko * 512:(ko + 1) * 512],
                        start=(k == 0), stop=False)
                    nc.tensor.matmul(
                        out=psum_rz[:, :],
                        lhsT=ht_sb[:, idx * KT + k:idx * KT + k + 1],
                        rhs=rzhh[hk][:, ko * 512:(ko + 1) * 512],
                        start=False, stop=False)
                    if ko == 3:
                        # keep-warm: re-run this resident chunk into the dummy
                        # bank so PE duty stays high while DMA-bound (a cold
                        # PE consumes slower than the stream and lags it)
                        for du_r in (rzih[hk], rzhh[hk]):
                            nc.tensor.matmul(
                                out=psum_du[:, :],
                                lhsT=xt_sb[:, k:k + 1],
                                rhs=du_r[:, 0:512],
                                start=True, stop=True, skip_group_check=True)
                nc.tensor.matmul(
                    out=psum_rz[:, :], lhsT=ones16[:, :],
                    rhs=bias_sb[:, idx * 1024:idx * 1024 + 512],
                    start=False, stop=True)

                # bias first: opens the accumulation group full-width so the
                # two lhsT-distinct halves can accumulate without their own
                # start=True (one PSUM group per bank region).
                nc.tensor.matmul(
                    out=psum_nn[:, :], lhsT=ones16[:, :],
                    rhs=bias_sb[:, idx * 1024 + 512:idx * 1024 + 1024],
                    start=True, stop=False)
                for k in range(KT):
                    hk, ko = divmod(k, 8)
                    nc.tensor.matmul(
                        out=psum_nn[:, 0:CH],
                        lhsT=xt_sb[:, k:k + 1],
                        rhs=nnih[hk][:, ko * CH:(ko + 1) * CH],
                        start=False, stop=False)
                    nc.tensor.matmul(
                        out=psum_nn[:, CH:2 * CH],
                        lhsT=ht_sb[:, idx * KT + k:idx * KT + k + 1],
                        rhs=nnhh[hk][:, ko * CH:(ko + 1) * CH],
                        start=False, stop=(k == KT - 1))
                    if ko == 7:
                        for du_r in (nnih[hk], nnhh[hk]):
                            nc.tensor.matmul(
                                out=psum_du[:, :],
                                lhsT=xt_sb[:, k:k + 1],
                                rhs=du_r[:, 0:512],
                                start=True, stop=True, skip_group_check=True)

                # ---- elementwise GRU tail (partition 0) ----
                rz_sb = wk.tile([1, 2 * CH], dt.float32, tag="rz_sb",
                                name=f"rz_sb_{idx}")
                nc.scalar.activation(rz_sb[:, :], psum_rz[:, :],
                                     mybir.ActivationFunctionType.Sigmoid)
                t1 = wk.tile([1, CH], dt.float32, tag="t1", name=f"t1_{idx}")
                nc.vector.tensor_mul(out=t1[:, :], in0=rz_sb[:, 0:CH],
                                     in1=psum_nn[:, CH:2 * CH])
                t2 = wk.tile([1, CH], dt.float32, tag="t2", name=f"t2_{idx}")
                nc.vector.tensor_add(out=t2[:, :], in0=t1[:, :],
                                     in1=psum_nn[:, 0:CH])
                n_sb = wk.tile([1, CH], dt.float32, tag="n_sb",
                               name=f"n_sb_{idx}")
                nc.scalar.activation(n_sb[:, :], t2[:, :],
                                     mybir.ActivationFunctionType.Tanh)
                mem_row = aux_sb[:, AUX_MEM + i * CH:AUX_MEM + (i + 1) * CH]
                res_row = aux_sb[:, AUX_RES + i * CH:AUX_RES + (i + 1) * CH]
                d_sb = wk.tile([1, CH], dt.float32, tag="d_sb",
                               name=f"d_sb_{idx}")
                nc.vector.tensor_sub(out=d_sb[:, :], in0=mem_row, in1=n_sb[:, :])
                u_sb = wk.tile([1, CH], dt.float32, tag="u_sb",
                               name=f"u_sb_{idx}")
                nc.vector.tensor_mul(out=u_sb[:, :], in0=d_sb[:, :],
                                     in1=rz_sb[:, CH:2 * CH])
                h_new = staging[:, OUT_MEM + i * CH:OUT_MEM + (i + 1) * CH]
                nc.vector.tensor_add(out=h_new, in0=u_sb[:, :], in1=n_sb[:, :])
                s_sb = wk.tile([1, CH], dt.float32, tag="s_sb",
                               name=f"s_sb_{idx}")
                nc.vector.tensor_add(out=s_sb[:, :], in0=h_new, in1=res_row)
                res_out = staging[:, OUT_RES + i * CH:OUT_RES + (i + 1) * CH]
                nc.scalar.activation(res_out, s_sb[:, :],
                                     mybir.ActivationFunctionType.Sigmoid)

                tr_mm(h_new,
                      aux_sb[:, AUX_CV + D + idx:AUX_CV + D + idx + 1])
                tr_mm(res_out,
                      aux_sb[:, AUX_CV + 2 * D + idx:AUX_CV + 2 * D + idx + 1])

            # ---- output Linear: partial_out = out_w[:, cols].T-packed @ comp ----
            compT = sp.tile([128, 2], dt.float16)
            nc.vector.tensor_copy(out=compT[:, :], in_=psum_tr[:, :])

            for j in range(4):
                psum_po = pp.tile([1, 512], dt.float32, tag="po",
                                  name=f"psum_po_{j}")
                for t in range(2):
                    nc.tensor.matmul(
                        out=psum_po[:, :],
                        lhsT=compT[:, t:t + 1],
                        rhs=ow_sb[t][:, j * 512:(j + 1) * 512],
                        start=(t == 0), stop=(t == 1))
                nc.vector.tensor_copy(out=staging[:, j * 512:(j + 1) * 512],
                                      in_=psum_po[:, :])

            nc.sync.dma_start(out=outbuf[:, :], in_=staging[:, :])

    nc.compile()
    return nc


def _fingerprint(*arrs):
    out = []
    for a in arrs:
        f = np.asarray(a).reshape(-1)
        stride = max(1, f.size // 64)
        out.append((a.shape, float(f[::stride].astype(np.float64).sum())))
    return tuple(out)


def _pack_block(block_f16):
    """[..., R, K] fp16 -> [..., 128, KT*R] moving-operand layout."""
    shp = block_f16.shape
    R, K = shp[-2], shp[-1]
    kt = K // 128
    bt = np.swapaxes(block_f16, -1, -2)           # [..., K, R]
    bt = bt.reshape(*shp[:-2], kt, 128, R)
    bt = np.swapaxes(bt, -3, -2)                  # [..., 128, kt, R]
    return np.ascontiguousarray(bt).reshape(*shp[:-2], 128, kt * R)


def _pack_weights(w_ih, w_hh, out_w, active):
    na = len(active)
    wpack = np.empty((NCORES, 128, na * F_LAYER + F_OW), F16)

    act = list(active)
    # gates split [3, NCORES, CH] over the 3H dim
    wi = w_ih.reshape(D, 3, NCORES, CH, I)[act].astype(F16)  # [na,3,NC,CH,I]
    wh = w_hh.reshape(D, 3, NCORES, CH, I)[act].astype(F16)
    rz_i = np.concatenate([wi[:, 0], wi[:, 1]], axis=2)      # [na,NC,512,I]
    rz_h = np.concatenate([wh[:, 0], wh[:, 1]], axis=2)
    nn_i = wi[:, 2]                                          # [na,NC,256,I]
    nn_h = wh[:, 2]
    p_rz_i = _pack_block(rz_i)                               # [na,NC,128,F_RZ]
    p_rz_h = _pack_block(rz_h)
    p_nn_i = _pack_block(nn_i)                               # [na,NC,128,F_NN]
    p_nn_h = _pack_block(nn_h)
    for a in range(na):
        base = a * F_LAYER
        wpack[:, :, base:base + F_RZ] = p_rz_i[a]
        wpack[:, :, base + F_RZ:base + 2 * F_RZ] = p_rz_h[a]
        wpack[:, :, base + 2 * F_RZ:base + 2 * F_RZ + F_NN] = p_nn_i[a]
        wpack[:, :, base + 2 * F_RZ + F_NN:base + F_LAYER] = p_nn_h[a]

    # out_w column slices: pack[c, p, t*O + n] = out_w[n, c*CH + t*128 + p]
    owt = out_w.astype(F16).reshape(O, NCORES, 2, 128)
    owt = np.ascontiguousarray(owt.transpose(1, 3, 2, 0))    # [NC,128,2,O]
    wpack[:, :, na * F_LAYER:] = owt.reshape(NCORES, 128, F_OW)
    return wpack


def _prepare(inputs):
    step = int(np.asarray(inputs["step"]))
    active = _active_layers(step)
    na = len(active)

    x = np.asarray(inputs["x"], F32)
    memory = np.asarray(inputs["memory"], F32)
    residual = np.asarray(inputs["residual"], F32)
    b_ih = np.asarray(inputs["b_ih"], F32)
    b_hh = np.asarray(inputs["b_hh"], F32)

    key = (active, _fingerprint(inputs["w_ih"], inputs["w_hh"],
                                inputs["out_w"]))
    if key not in _PACK_CACHE:
        _PACK_CACHE.clear()
        _PACK_CACHE[key] = _pack_weights(
            np.asarray(inputs["w_ih"], F32), np.asarray(inputs["w_hh"], F32),
            np.asarray(inputs["out_w"], F32), active)
    wpack = _PACK_CACHE[key]

    xt = np.ascontiguousarray(x.astype(F16).reshape(KT, 128).T)
    ht = np.ascontiguousarray(
        memory[list(active)].astype(F16).reshape(na, KT, 128)
        .transpose(2, 0, 1)).reshape(128, na * KT)

    bsum = (b_ih + b_hh).reshape(D, 3, NCORES, CH)
    bi = b_ih.reshape(D, 3, NCORES, CH)
    bh = b_hh.reshape(D, 3, NCORES, CH)
    bias16 = np.empty((NCORES, 1, na * 1024), F16)
    for a, i in enumerate(active):
        bias16[:, 0, a * 1024:a * 1024 + 256] = bsum[i, 0].astype(F16)
        bias16[:, 0, a * 1024 + 256:a * 1024 + 512] = bsum[i, 1].astype(F16)
        bias16[:, 0, a * 1024 + 512:a * 1024 + 768] = bi[i, 2].astype(F16)
        bias16[:, 0, a * 1024 + 768:a * 1024 + 1024] = bh[i, 2].astype(F16)

    aux32 = np.zeros((NCORES, 1, AUX_LEN), F32)
    aux32[:, 0, AUX_MEM:AUX_MEM + D * CH] = (
        memory.reshape(D, NCORES, CH).transpose(1, 0, 2).reshape(NCORES, -1))
    aux32[:, 0, AUX_RES:AUX_RES + D * CH] = (
        residual.reshape(D, NCORES, CH).transpose(1, 0, 2).reshape(NCORES, -1))
    for a, i in enumerate(active):
        aux32[:, 0, AUX_CV + a] = 1.0 / (i + 1) ** 2          # fused weight
        aux32[:, 0, AUX_CV + D + a] = 1.0 / (a + 1) ** 2      # positional
        aux32[:, 0, AUX_CV + 2 * D + a] = 1.0 / (i + 1) ** 2  # residual
    inactive = [i for i in range(D) if i not in active]
    if inactive:
        base = np.zeros(H, F32)
        for i in inactive:
            base += residual[i] / (i + 1) ** 2
        aux32[:, 0, AUX_BASE:AUX_BASE + CH] = base.reshape(NCORES, CH)

    in_maps = [
        {"wpack": wpack[c], "xt": xt, "ht": ht,
         "bias16": bias16[c], "aux32": aux32[c]}
        for c in range(NCORES)
    ]
    return active, in_maps


def _assemble(inputs, active, per_core):
    memory = np.asarray(inputs["memory"], F32)
    residual = np.asarray(inputs["residual"], F32)
    out_b = np.asarray(inputs["out_b"], F32)

    stacked = np.stack([per_core[c][0] for c in range(NCORES)])  # [NC, OUT_LEN]
    output = stacked[:, OUT_P:OUT_P + O].sum(axis=0) + out_b
    new_mem = memory.copy()
    new_res = residual.copy()
    for c in range(NCORES):
        sl = slice(c * CH, (c + 1) * CH)
        for i in active:
            new_mem[i, sl] = stacked[c, OUT_MEM + i * CH:OUT_MEM + (i + 1) * CH]
            new_res[i, sl] = stacked[c, OUT_RES + i * CH:OUT_RES + (i + 1) * CH]
    return output, new_res, new_mem


def _execute(inputs, trace=False, **kwargs):
    active, in_maps = _prepare(inputs)
    if active not in _KERNEL_CACHE:
        _KERNEL_CACHE[active] = _build_nc(active)
    nc = _KERNEL_CACHE[active]
    try:
        res = run_bass_kernel_spmd(nc, in_maps, list(range(NCORES)),
                                   trace=trace, **kwargs)
    except Exception:
        # The first execution of a freshly compiled NEFF under the NTFF
        # profiler is flaky (NRT_EXEC_UNIT_UNRECOVERABLE); one retry after
        # the warm load consistently succeeds.
        import time as _time
        _time.sleep(2.0)
        res = run_bass_kernel_spmd(nc, in_maps, list(range(NCORES)),
                                   trace=trace, **kwargs)
    per_core = [res.results[c]["outbuf"] for c in range(NCORES)]
    return _assemble(inputs, active, per_core), res


def kernel(**inputs):
    outs, _ = _execute(inputs)
    return outs


def kernel_profiled(inputs, warmup=True, **kwargs):
    if warmup:
        _execute(inputs, trace=False)
    outs, res = _execute(inputs, trace=True, **kwargs)
    return outs, res


# revision 22
# speedup vs baseline: 1.0394x; 1.0394x over previous
"""Trainium2 Bass kernel for nn_DialatedRNN (D=4, I=H=O=2048, batch=1).

Strategy (tensor-parallel over 8 NeuronCores):
  - Each core owns a 256-wide slice of H for every gate (r/z/n) of every
    active GRU layer, plus a 256-wide column slice of the output Linear.
  - Gate matvecs run on the TensorEngine with the (tiny) x / h vectors as
    the *stationary* operand and the fp16-packed transposed weights as the
    *moving* operand, accumulating over 16 K-tiles into PSUM.  Biases are
    folded in with one extra K=1 matmul per gate group.
  - The elementwise GRU tail (sigmoid/tanh/blend) runs on DVE+ACT on
    partition 0; the composite vector is accumulated *in column layout*
    directly in PSUM via K=1/M=128 matmuls so the final out_w matvec needs
    no transpose.
  - Each core returns one packed [1, 4096] f32 buffer:
    [ partial_out(2048) | new_mem(4x256) | new_res(4x256) ].
    The host sums the 8 partial outputs, adds out_b, and reassembles the
    H-sharded new_res / new_mem.

All weights are streamed as fp16 (f32 accumulate in PSUM): ~26 MB per core,
which is the memory-roofline-relevant traffic.  End-to-end rel-err vs the
f32 reference is ~3e-4.
"""

import sys
import types

import numpy as np

import concourse.bacc as bacc
import concourse.mybir as mybir
import concourse.tile as tile
from concourse.bass_utils import run_bass_kernel_spmd


def _ensure_ntff_hook():
    """The agent image's ``antenv`` package lacks ``axon_hooks``, which
    ``run_bass_kernel_spmd(trace=True)`` imports under axon.  Provide the
    tiny get/set module and register the same ctypes-based NTFF hook the
    axon boot would have installed, so profiling works."""
    try:
        from antenv.axon_hooks import get_axon_ntff_profile_hook  # noqa: F401
        return
    except ImportError:
        pass
    mod = types.ModuleType("antenv.axon_hooks")
    holder = {"hook": None}
    mod.set_axon_ntff_profile_hook = lambda h: holder.__setitem__("hook", h)
    mod.get_axon_ntff_profile_hook = lambda: holder["hook"]
    try:
        import antenv
        sys.modules["antenv.axon_hooks"] = mod
        antenv.axon_hooks = mod
        from trn_agent_boot.trn_boot import _ntff_profile_via_ctypes
        hook = _ntff_profile_via_ctypes("/opt/axon/libaxon_pjrt.so")
        if hook is not None:
            mod.set_axon_ntff_profile_hook(hook)
    except Exception:
        pass


_ensure_ntff_hook()

F16 = np.float16
F32 = np.float32

D, I, H, O = 4, 2048, 2048, 2048
NCORES = 8
CH = H // NCORES            # 256: per-core slice of H (and of O columns)
KT = I // 128               # 16 K-tiles for the gate contractions
F_RZ = KT * 2 * CH          # 8192 fp16 elems/partition for a packed rz block
F_NN = KT * CH              # 4096 for a packed n block
F_LAYER = 2 * F_RZ + 2 * F_NN   # 24576
F_OW = 2 * O                # 4096: out_w column-slice, 2 K-tiles x 2048

AUX_MEM = 0                 # [D*CH]   f32 memory rows (per-core slice)
AUX_RES = D * CH            # [D*CH]   f32 residual rows
AUX_CV = 2 * D * CH         # [3*D]    composite scales (fused | cpos | cres)
AUX_BASE = AUX_CV + 3 * D   # [CH]     base composite (inactive layers)
AUX_LEN = AUX_BASE + CH

OUT_P = 0                   # packed output layout
OUT_MEM = O
OUT_RES = O + D * CH
OUT_LEN = O + 2 * D * CH    # 4096

_KERNEL_CACHE: dict = {}
_PACK_CACHE: dict = {}


def _active_layers(step: int):
    return tuple(i for i in range(D) if step % ((i + 1) ** 2) == 0)


def _build_nc(active):
    """Build + compile the per-core SPMD Bass program for a given active set."""
    na = len(active)
    has_inactive = na < D
    # per active layer: does the positional weight equal the residual weight?
    fuse = tuple(1.0 / (a + 1) ** 2 == 1.0 / (i + 1) ** 2
                 for a, i in enumerate(active))

    nc = bacc.Bacc("TRN2", target_bir_lowering=False, debug=False)
    dt = mybir.dt

    wpack = nc.dram_tensor("wpack", [128, na * F_LAYER + F_OW], dt.float16,
                           kind="ExternalInput").ap()
    xt = nc.dram_tensor("xt", [128, KT], dt.float16, kind="ExternalInput").ap()
    ht = nc.dram_tensor("ht", [128, na * KT], dt.float16,
                        kind="ExternalInput").ap()
    bias16 = nc.dram_tensor("bias16", [1, na * 1024], dt.float16,
                            kind="ExternalInput").ap()
    aux32 = nc.dram_tensor("aux32", [1, AUX_LEN], dt.float32,
                           kind="ExternalInput").ap()
    outbuf = nc.dram_tensor("outbuf", [1, OUT_LEN], dt.float32,
                            kind="ExternalOutput").ap()

    with tile.TileContext(nc) as tc:
        with (
            tc.tile_pool(name="wp", bufs=4) as wp,
            tc.tile_pool(name="sp", bufs=1) as sp,
            tc.tile_pool(name="wk", bufs=2) as wk,
            tc.tile_pool(name="pp", bufs=2, space="PSUM") as pp,
        ):
            # Small inputs ride SWDGE (gpsimd) so the HWDGE rings start
            # streaming weights immediately instead of paying 4 serialized
            # ~2.6us fixed costs first.
            xt_sb = sp.tile([128, KT], dt.float16)
            nc.gpsimd.dma_start(out=xt_sb[:, :], in_=xt[:, :])
            ht_sb = sp.tile([128, na * KT], dt.float16)
            nc.gpsimd.dma_start(out=ht_sb[:, :], in_=ht[:, :])
            bias_sb = sp.tile([1, na * 1024], dt.float16)
            nc.gpsimd.dma_start(out=bias_sb[:, :], in_=bias16[:, :])
            aux_sb = sp.tile([1, AUX_LEN], dt.float32)
            nc.gpsimd.dma_start(out=aux_sb[:, :], in_=aux32[:, :])

            ones16 = sp.tile([1, 1], dt.float16)
            nc.vector.memset(ones16[:, :], 1.0)
            one32 = sp.tile([1, 1], dt.float32)
            nc.vector.memset(one32[:, :], 1.0)

            staging = sp.tile([1, OUT_LEN], dt.float32)
            if has_inactive:
                nc.vector.memset(staging[:, :], 0.0)

            # out_w slice rides SWDGE early: keeps its 2MB off the HWDGE
            # gate-weight stream (whose end time bounds the serial tail) and
            # it is resident long before the final matvec needs it.
            ow_sb = []
            for t in range(2):
                t_ow = wp.tile([128, O], dt.float16, tag="ow", bufs=2,
                               name=f"ow_{t}")
                nc.gpsimd.dma_start(
                    out=t_ow[:, :],
                    in_=wpack[:, na * F_LAYER + t * O:na * F_LAYER + (t + 1) * O])
                ow_sb.append(t_ow)

            psum_tr = pp.tile([128, 2], dt.float32, tag="tr", bufs=1)
            n_contrib = (1 if has_inactive else 0) + 2 * na
            contrib = [0]  # mutable counter shared across emitters

            # Round-robin weight DMAs over both HWDGE rings (SP + ACT).
            dma_rr = [0]

            def wdma(out_ap, off, flen):
                eng = nc.sync if dma_rr[0] % 2 == 0 else nc.scalar
                dma_rr[0] += 1
                eng.dma_start(out=out_ap, in_=wpack[:, off:off + flen])

            def tr_mm(row_ap, scale_ap):
                """psum_tr[:, t] += row[t*128:(t+1)*128].T * scale, t=0,1.

                start=True clears the PSUM tile's whole bank region, so only
                the very first matmul may carry it; later columns accumulate
                onto the cleared zeros.
                """
                for t in range(2):
                    nc.tensor.matmul(
                        out=psum_tr[:, t:t + 1],
                        lhsT=row_ap[:, t * 128:(t + 1) * 128],
                        rhs=scale_ap,
                        start=(contrib[0] == 0 and t == 0),
                        stop=(contrib[0] == n_contrib - 1 and t == 1),
                        skip_group_check=True,
                    )
                contrib[0] += 1

            if has_inactive:
                tr_mm(aux_sb[:, AUX_BASE:AUX_BASE + CH], one32[:, :])

            for idx, i in enumerate(active):
                base = idx * F_LAYER
                rzih = []
                rzhh = []
                nnih = []
                nnhh = []
                # 512KB chunks, interleaved in exactly the order the PE
                # consumes them: rz quarters (4 k-tiles each), nn halves.
                for q in range(4):
                    t_rzih = wp.tile([128, F_RZ // 4], dt.float16, tag="rzih",
                                     bufs=8, name=f"rzih_{idx}_{q}")
                    wdma(t_rzih[:, :], base + q * F_RZ // 4, F_RZ // 4)
                    rzih.append(t_rzih)
                    t_rzhh = wp.tile([128, F_RZ // 4], dt.float16, tag="rzhh",
                                     bufs=8, name=f"rzhh_{idx}_{q}")
                    wdma(t_rzhh[:, :], base + F_RZ + q * F_RZ // 4, F_RZ // 4)
                    rzhh.append(t_rzhh)
                for h in range(2):
                    t_nnih = wp.tile([128, F_NN // 2], dt.float16, tag="nnih",
                                     bufs=4, name=f"nnih_{idx}_{h}")
                    wdma(t_nnih[:, :], base + 2 * F_RZ + h * F_NN // 2,
                         F_NN // 2)
                    nnih.append(t_nnih)
                    t_nnhh = wp.tile([128, F_NN // 2], dt.float16, tag="nnhh",
                                     bufs=4, name=f"nnhh_{idx}_{h}")
                    wdma(t_nnhh[:, :], base + 2 * F_RZ + F_NN + h * F_NN // 2,
                         F_NN // 2)
                    nnhh.append(t_nnhh)

                psum_rz = pp.tile([1, 2 * CH], dt.float32, tag="rz",
                                  name=f"psum_rz_{idx}")
                psum_nn = pp.tile([1, 2 * CH], dt.float32, tag="nn",
                                  name=f"psum_nn_{idx}")

                for k in range(KT):
                    hk, ko = divmod(k, 4)
                    nc.tensor.matmul(
                        out=psum_rz[:, :],
                        lhsT=xt_sb[:, k:k + 1],
                        rhs=rzih[hk][:, ko * 512:(ko + 1) * 512],
                        start=(k == 0), stop=False)
                    nc.tensor.matmul(
                        out=psum_rz[:, :],
                        lhsT=ht_sb[:, idx * KT + k:idx * KT + k + 1],
                        rhs=rzhh[hk][:, ko * 512:(ko + 1) * 512],
                        start=False, stop=False)
                nc.tensor.matmul(
                    out=psum_rz[:, :], lhsT=ones16[:, :],
                    rhs=bias_sb[:, idx * 1024:idx * 1024 + 512],
                    start=False, stop=True)

                # bias first: opens the accumulation group full-width so the
                # two lhsT-distinct halves can accumulate without their own
                # start=True (one PSUM group per bank region).
                nc.tensor.matmul(
                    out=psum_nn[:, :], lhsT=ones16[:, :],
                    rhs=bias_sb[:, idx * 1024 + 512:idx * 1024 + 1024],
                    start=True, stop=False)
                for k in range(KT):
                    hk, ko = divmod(k, 8)
                    nc.tensor.matmul(
                        out=psum_nn[:, 0:CH],
                        lhsT=xt_sb[:, k:k + 1],
                        rhs=nnih[hk][:, ko * CH:(ko + 1) * CH],
                        start=False, stop=False)
                    nc.tensor.matmul(
                        out=psum_nn[:, CH:2 * CH],
                        lhsT=ht_sb[:, idx * KT + k:idx * KT + k + 1],
                        rhs=nnhh[hk][:, ko * CH:(ko + 1) * CH],
                        start=False, stop=(k == KT - 1))

                # ---- elementwise GRU tail (partition 0) ----
                rz_sb = wk.tile([1, 2 * CH], dt.float32, tag="rz_sb",
                                name=f"rz_sb_{idx}")
                nc.scalar.activation(rz_sb[:, :], psum_rz[:, :],
                                     mybir.ActivationFunctionType.Sigmoid)
                t1 = wk.tile([1, CH], dt.float32, tag="t1", name=f"t1_{idx}")
                nc.vector.tensor_mul(out=t1[:, :], in0=rz_sb[:, 0:CH],
                                     in1=psum_nn[:, CH:2 * CH])
                t2 = wk.tile([1, CH], dt.float32, tag="t2", name=f"t2_{idx}")
                nc.vector.tensor_add(out=t2[:, :], in0=t1[:, :],
                                     in1=psum_nn[:, 0:CH])
                n_sb = wk.tile([1, CH], dt.float32, tag="n_sb",
                               name=f"n_sb_{idx}")
                nc.scalar.activation(n_sb[:, :], t2[:, :],
                                     mybir.ActivationFunctionType.Tanh)
                mem_row = aux_sb[:, AUX_MEM + i * CH:AUX_MEM + (i + 1) * CH]
                res_row = aux_sb[:, AUX_RES + i * CH:AUX_RES + (i + 1) * CH]
                d_sb = wk.tile([1, CH], dt.float32, tag="d_sb",
                               name=f"d_sb_{idx}")
                nc.vector.tensor_sub(out=d_sb[:, :], in0=mem_row, in1=n_sb[:, :])
                u_sb = wk.tile([1, CH], dt.float32, tag="u_sb",
                               name=f"u_sb_{idx}")
                nc.vector.tensor_mul(out=u_sb[:, :], in0=d_sb[:, :],
                                     in1=rz_sb[:, CH:2 * CH])
                h_new = staging[:, OUT_MEM + i * CH:OUT_MEM + (i + 1) * CH]
                nc.vector.tensor_add(out=h_new, in0=u_sb[:, :], in1=n_sb[:, :])
                s_sb = wk.tile([1, CH], dt.float32, tag="s_sb",
                               name=f"s_sb_{idx}")
                nc.vector.tensor_add(out=s_sb[:, :], in0=h_new, in1=res_row)
                res_out = staging[:, OUT_RES + i * CH:OUT_RES + (i + 1) * CH]
                nc.scalar.activation(res_out, s_sb[:, :],
                                     mybir.ActivationFunctionType.Sigmoid)

                tr_mm(h_new,
                      aux_sb[:, AUX_CV + D + idx:AUX_CV + D + idx + 1])
                tr_mm(res_out,
                      aux_sb[:, AUX_CV + 2 * D + idx:AUX_CV + 2 * D + idx + 1])

            # ---- output Linear: partial_out = out_w[:, cols].T-packed @ comp ----
            compT = sp.tile([128, 2], dt.float16)
            nc.vector.tensor_copy(out=compT[:, :], in_=psum_tr[:, :])

            for j in range(4):
                psum_po = pp.tile([1, 512], dt.float32, tag="po",
                                  name=f"psum_po_{j}")
                for t in range(2):
                    nc.tensor.matmul(
                        out=psum_po[:, :],
                        lhsT=compT[:, t:t + 1],
                        rhs=ow_sb[t][:, j * 512:(j + 1) * 512],
                        start=(t == 0), stop=(t == 1))
                nc.vector.tensor_copy(out=staging[:, j * 512:(j + 1) * 512],
                                      in_=psum_po[:, :])

            nc.sync.dma_start(out=outbuf[:, :], in_=staging[:, :])

    nc.compile()
    return nc


def _fingerprint(*arrs):
    out = []
    for a in arrs:
        f = np.asarray(a).reshape(-1)
        stride = max(1, f.size // 64)
        out.append((a.shape, float(f[::stride].astype(np.float64).sum())))
    return tuple(out)


def _pack_block(block_f16):
    """[..., R, K] fp16 -> [..., 128, KT*R] moving-operand layout."""
    shp = block_f16.shape
    R, K = shp[-2], shp[-1]
    kt = K // 128
    bt = np.swapaxes(block_f16, -1, -2)           # [..., K, R]
    bt = bt.reshape(*shp[:-2], kt, 128, R)
    bt = np.swapaxes(bt, -3, -2)                  # [..., 128, kt, R]
    return np.ascontiguousarray(bt).reshape(*shp[:-2], 128, kt * R)


def _pack_weights(w_ih, w_hh, out_w, active):
    na = len(active)
    wpack = np.empty((NCORES, 128, na * F_LAYER + F_OW), F16)

    act = list(active)
    # gates split [3, NCORES, CH] over the 3H dim
    wi = w_ih.reshape(D, 3, NCORES, CH, I)[act].astype(F16)  # [na,3,NC,CH,I]
    wh = w_hh.reshape(D, 3, NCORES, CH, I)[act].astype(F16)
    rz_i = np.concatenate([wi[:, 0], wi[:, 1]], axis=2)      # [na,NC,512,I]
    rz_h = np.concatenate([wh[:, 0], wh[:, 1]], axis=2)
    nn_i = wi[:, 2]                                          # [na,NC,256,I]
    nn_h = wh[:, 2]
    p_rz_i = _pack_block(rz_i)                               # [na,NC,128,F_RZ]
    p_rz_h = _pack_block(rz_h)
    p_nn_i = _pack_block(nn_i)                               # [na,NC,128,F_NN]
    p_nn_h = _pack_block(nn_h)
    for a in range(na):
        base = a * F_LAYER
        wpack[:, :, base:base + F_RZ] = p_rz_i[a]
        wpack[:, :, base + F_RZ:base + 2 * F_RZ] = p_rz_h[a]
        wpack[:, :, base + 2 * F_RZ:base + 2 * F_RZ + F_NN] = p_nn_i[a]
        wpack[:, :, base + 2 * F_RZ + F_NN:base + F_LAYER] = p_nn_h[a]

    # out_w column slices: pack[c, p, t*O + n] = out_w[n, c*CH + t*128 + p]
    owt = out_w.astype(F16).reshape(O, NCORES, 2, 128)
    owt = np.ascontiguousarray(owt.transpose(1, 3, 2, 0))    # [NC,128,2,O]
    wpack[:, :, na * F_LAYER:] = owt.reshape(NCORES, 128, F_OW)
    return wpack


def _prepare(inputs):
    step = int(np.asarray(inputs["step"]))
    active = _active_layers(step)
    na = len(active)

    x = np.asarray(inputs["x"], F32)
    memory = np.asarray(inputs["memory"], F32)
    residual = np.asarray(inputs["residual"], F32)
    b_ih = np.asarray(inputs["b_ih"], F32)
    b_hh = np.asarray(inputs["b_hh"], F32)

    key = (active, _fingerprint(inputs["w_ih"], inputs["w_hh"],
                                inputs["out_w"]))
    if key not in _PACK_CACHE:
        _PACK_CACHE.clear()
        _PACK_CACHE[key] = _pack_weights(
            np.asarray(inputs["w_ih"], F32), np.asarray(inputs["w_hh"], F32),
            np.asarray(inputs["out_w"], F32), active)
    wpack = _PACK_CACHE[key]

    xt = np.ascontiguousarray(x.astype(F16).reshape(KT, 128).T)
    ht = np.ascontiguousarray(
        memory[list(active)].astype(F16).reshape(na, KT, 128)
        .transpose(2, 0, 1)).reshape(128, na * KT)

    bsum = (b_ih + b_hh).reshape(D, 3, NCORES, CH)
    bi = b_ih.reshape(D, 3, NCORES, CH)
    bh = b_hh.reshape(D, 3, NCORES, CH)
    bias16 = np.empty((NCORES, 1, na * 1024), F16)
    for a, i in enumerate(active):
        bias16[:, 0, a * 1024:a * 1024 + 256] = bsum[i, 0].astype(F16)
        bias16[:, 0, a * 1024 + 256:a * 1024 + 512] = bsum[i, 1].astype(F16)
        bias16[:, 0, a * 1024 + 512:a * 1024 + 768] = bi[i, 2].astype(F16)
        bias16[:, 0, a * 1024 + 768:a * 1024 + 1024] = bh[i, 2].astype(F16)

    aux32 = np.zeros((NCORES, 1, AUX_LEN), F32)
    aux32[:, 0, AUX_MEM:AUX_MEM + D * CH] = (
        memory.reshape(D, NCORES, CH).transpose(1, 0, 2).reshape(NCORES, -1))
    aux32[:, 0, AUX_RES:AUX_RES + D * CH] = (
        residual.reshape(D, NCORES, CH).transpose(1, 0, 2).reshape(NCORES, -1))
    for a, i in enumerate(active):
        aux32[:, 0, AUX_CV + a] = 1.0 / (i + 1) ** 2          # fused weight
        aux32[:, 0, AUX_CV + D + a] = 1.0 / (a + 1) ** 2      # positional
        aux32[:, 0, AUX_CV + 2 * D + a] = 1.0 / (i + 1) ** 2  # residual
    inactive = [i for i in range(D) if i not in active]
    if inactive:
        base = np.zeros(H, F32)
        for i in inactive:
            base += residual[i] / (i + 1) ** 2
        aux32[:, 0, AUX_BASE:AUX_BASE + CH] = base.reshape(NCORES, CH)

    in_maps = [
        {"wpack": wpack[c], "xt": xt, "ht": ht,
         "bias16": bias16[c], "aux32": aux32[c]}
        for c in range(NCORES)
    ]
    return active, in_maps


def _assemble(inputs, active, per_core):
    memory = np.asarray(inputs["memory"], F32)
    residual = np.asarray(inputs["residual"], F32)
    out_b = np.asarray(inputs["out_b"], F32)

    stacked = np.stack([per_core[c][0] for c in range(NCORES)])  # [NC, OUT_LEN]
    output = stacked[:, OUT_P:OUT_P + O].sum(axis=0) + out_b
    new_mem = memory.copy()
    new_res = residual.copy()
    for c in range(NCORES):
        sl = slice(c * CH, (c + 1) * CH)
        for i in active:
            new_mem[i, sl] = stacked[c, OUT_MEM + i * CH:OUT_MEM + (i + 1) * CH]
            new_res[i, sl] = stacked[c, OUT_RES + i * CH:OUT_RES + (i + 1) * CH]
    return output, new_res, new_mem


def _execute(inputs, trace=False, **kwargs):
    active, in_maps = _prepare(inputs)
    if active not in _KERNEL_CACHE:
        _KERNEL_CACHE[active] = _build_nc(active)
    nc = _KERNEL_CACHE[active]
    try:
        res = run_bass_kernel_spmd(nc, in_maps, list(range(NCORES)),
                                   trace=trace, **kwargs)
    except Exception:
        # The first execution of a freshly compiled NEFF under the NTFF
        # profiler is flaky (NRT_EXEC_UNIT_UNRECOVERABLE); one retry after
        # the warm load consistently succeeds.
        import time as _time
        _time.sleep(2.0)
        res = run_bass_kernel_spmd(nc, in_maps, list(range(NCORES)),
                                   trace=trace, **kwargs)
    per_core = [res.results[c]["outbuf"] for c in range(NCORES)]
    return _assemble(inputs, active, per_core), res


def kernel(**inputs):
    outs, _ = _execute(inputs)
    return outs


def kernel_profiled(inputs, warmup=True, **kwargs):
    if warmup:
        _execute(inputs, trace=False)
    outs, res = _execute(inputs, trace=True, **kwargs)
    return outs, res


# revision 23
# speedup vs baseline: 1.1444x; 1.1010x over previous
"""Trainium2 Bass kernel for nn_DialatedRNN (D=4, I=H=O=2048, batch=1).

Strategy (tensor-parallel over 8 NeuronCores):
  - Each core owns a 256-wide slice of H for every gate (r/z/n) of every
    active GRU layer, plus a 256-wide column slice of the output Linear.
  - Gate matvecs run on the TensorEngine with the (tiny) x / h vectors as
    the *stationary* operand and the fp16-packed transposed weights as the
    *moving* operand, accumulating over 16 K-tiles into PSUM.  Biases are
    folded in with one extra K=1 matmul per gate group.
  - The elementwise GRU tail (sigmoid/tanh/blend) runs on DVE+ACT on
    partition 0; the composite vector is accumulated *in column layout*
    directly in PSUM via K=1/M=128 matmuls so the final out_w matvec needs
    no transpose.
  - Each core returns one packed [1, 4096] f32 buffer:
    [ partial_out(2048) | new_mem(4x256) | new_res(4x256) ].
    The host sums the 8 partial outputs, adds out_b, and reassembles the
    H-sharded new_res / new_mem.

All weights are streamed as fp16 (f32 accumulate in PSUM): ~26 MB per core,
which is the memory-roofline-relevant traffic.  End-to-end rel-err vs the
f32 reference is ~3e-4.
"""

import sys
import types

import numpy as np

import concourse.bacc as bacc
import concourse.mybir as mybir
import concourse.tile as tile
from concourse.bass_utils import run_bass_kernel_spmd


def _ensure_ntff_hook():
    """The agent image's ``antenv`` package lacks ``axon_hooks``, which
    ``run_bass_kernel_spmd(trace=True)`` imports under axon.  Provide the
    tiny get/set module and register the same ctypes-based NTFF hook the
    axon boot would have installed, so profiling works."""
    try:
        from antenv.axon_hooks import get_axon_ntff_profile_hook  # noqa: F401
        return
    except ImportError:
        pass
    mod = types.ModuleType("antenv.axon_hooks")
    holder = {"hook": None}
    mod.set_axon_ntff_profile_hook = lambda h: holder.__setitem__("hook", h)
    mod.get_axon_ntff_profile_hook = lambda: holder["hook"]
    try:
        import antenv
        sys.modules["antenv.axon_hooks"] = mod
        antenv.axon_hooks = mod
        from trn_agent_boot.trn_boot import _ntff_profile_via_ctypes
        hook = _ntff_profile_via_ctypes("/opt/axon/libaxon_pjrt.so")
        if hook is not None:
            mod.set_axon_ntff_profile_hook(hook)
    except Exception:
        pass


_ensure_ntff_hook()

F16 = np.float16
F32 = np.float32

D, I, H, O = 4, 2048, 2048, 2048
NCORES = 8
CH = H // NCORES            # 256: per-core slice of H (and of O columns)
KT = I // 128               # 16 K-tiles for the gate contractions
F_RZ = KT * 2 * CH          # 8192 fp16 elems/partition for a packed rz block
F_NN = KT * CH              # 4096 for a packed n block
F_LAYER = 2 * F_RZ + 2 * F_NN   # 24576
F_OW = 2 * O                # 4096: out_w column-slice, 2 K-tiles x 2048

AUX_MEM = 0                 # [D*CH]   f32 memory rows (per-core slice)
AUX_RES = D * CH            # [D*CH]   f32 residual rows
AUX_CV = 2 * D * CH         # [3*D]    composite scales (fused | cpos | cres)
AUX_BASE = AUX_CV + 3 * D   # [CH]     base composite (inactive layers)
AUX_LEN = AUX_BASE + CH

OUT_P = 0                   # packed output layout
OUT_MEM = O
OUT_RES = O + D * CH
OUT_LEN = O + 2 * D * CH    # 4096

_KERNEL_CACHE: dict = {}
_PACK_CACHE: dict = {}


def _active_layers(step: int):
    return tuple(i for i in range(D) if step % ((i + 1) ** 2) == 0)


def _build_nc(active):
    """Build + compile the per-core SPMD Bass program for a given active set."""
    na = len(active)
    has_inactive = na < D
    # per active layer: does the positional weight equal the residual weight?
    fuse = tuple(1.0 / (a + 1) ** 2 == 1.0 / (i + 1) ** 2
                 for a, i in enumerate(active))

    nc = bacc.Bacc("TRN2", target_bir_lowering=False, debug=False)
    dt = mybir.dt

    wpack = nc.dram_tensor("wpack", [128, na * F_LAYER + F_OW], dt.float16,
                           kind="ExternalInput").ap()
    xt = nc.dram_tensor("xt", [128, KT], dt.float16, kind="ExternalInput").ap()
    ht = nc.dram_tensor("ht", [128, na * KT], dt.float16,
                        kind="ExternalInput").ap()
    bias16 = nc.dram_tensor("bias16", [1, na * 1024], dt.float16,
                            kind="ExternalInput").ap()
    aux32 = nc.dram_tensor("aux32", [1, AUX_LEN], dt.float32,
                           kind="ExternalInput").ap()
    outbuf = nc.dram_tensor("outbuf", [1, OUT_LEN], dt.float32,
                            kind="ExternalOutput").ap()

    with tile.TileContext(nc) as tc:
        with (
            tc.tile_pool(name="wp", bufs=4) as wp,
            tc.tile_pool(name="sp", bufs=1) as sp,
            tc.tile_pool(name="wk", bufs=2) as wk,
            tc.tile_pool(name="pp", bufs=2, space="PSUM") as pp,
        ):
            # Small inputs ride SWDGE (gpsimd) so the HWDGE rings start
            # streaming weights immediately instead of paying 4 serialized
            # ~2.6us fixed costs first.
            xt_sb = sp.tile([128, KT], dt.float16)
            nc.gpsimd.dma_start(out=xt_sb[:, :], in_=xt[:, :])
            ht_sb = sp.tile([128, na * KT], dt.float16)
            nc.gpsimd.dma_start(out=ht_sb[:, :], in_=ht[:, :])
            bias_sb = sp.tile([1, na * 1024], dt.float16)
            nc.gpsimd.dma_start(out=bias_sb[:, :], in_=bias16[:, :])
            aux_sb = sp.tile([1, AUX_LEN], dt.float32)
            nc.gpsimd.dma_start(out=aux_sb[:, :], in_=aux32[:, :])

            ones16 = sp.tile([1, 1], dt.float16)
            nc.vector.memset(ones16[:, :], 1.0)
            one32 = sp.tile([1, 1], dt.float32)
            nc.vector.memset(one32[:, :], 1.0)

            staging = sp.tile([1, OUT_LEN], dt.float32)
            if has_inactive:
                nc.vector.memset(staging[:, :], 0.0)

            # out_w slice rides SWDGE early: keeps its 2MB off the HWDGE
            # gate-weight stream (whose end time bounds the serial tail) and
            # it is resident long before the final matvec needs it.
            ow_sb = []
            for t in range(2):
                t_ow = wp.tile([128, O], dt.float16, tag="ow", bufs=2,
                               name=f"ow_{t}")
                nc.gpsimd.dma_start(
                    out=t_ow[:, :],
                    in_=wpack[:, na * F_LAYER + t * O:na * F_LAYER + (t + 1) * O])
                ow_sb.append(t_ow)

            psum_tr = pp.tile([128, 2], dt.float32, tag="tr", bufs=1)
            n_contrib = (1 if has_inactive else 0) + 2 * na
            contrib = [0]  # mutable counter shared across emitters

            # Round-robin weight DMAs over both HWDGE rings (SP + ACT).
            dma_rr = [0]

            def wdma(out_ap, off, flen):
                # sync ring ONLY: issuing from nc.scalar interleaves DMA
                # triggers with the per-layer activations in ACT's program,
                # serializing postproc against next-layer DMA issue.
                dma_rr[0] += 1
                nc.sync.dma_start(out=out_ap, in_=wpack[:, off:off + flen])

            def tr_mm(row_ap, scale_ap):
                """psum_tr[:, t] += row[t*128:(t+1)*128].T * scale, t=0,1.

                start=True clears the PSUM tile's whole bank region, so only
                the very first matmul may carry it; later columns accumulate
                onto the cleared zeros.
                """
                for t in range(2):
                    nc.tensor.matmul(
                        out=psum_tr[:, t:t + 1],
                        lhsT=row_ap[:, t * 128:(t + 1) * 128],
                        rhs=scale_ap,
                        start=(contrib[0] == 0 and t == 0),
                        stop=(contrib[0] == n_contrib - 1 and t == 1),
                        skip_group_check=True,
                    )
                contrib[0] += 1

            if has_inactive:
                tr_mm(aux_sb[:, AUX_BASE:AUX_BASE + CH], one32[:, :])

            for idx, i in enumerate(active):
                base = idx * F_LAYER
                rzih = []
                rzhh = []
                nnih = []
                nnhh = []
                # 512KB chunks, interleaved in exactly the order the PE
                # consumes them: rz quarters (4 k-tiles each), nn halves.
                for q in range(4):
                    t_rzih = wp.tile([128, F_RZ // 4], dt.float16, tag="rzih",
                                     bufs=8, name=f"rzih_{idx}_{q}")
                    wdma(t_rzih[:, :], base + q * F_RZ // 4, F_RZ // 4)
                    rzih.append(t_rzih)
                    t_rzhh = wp.tile([128, F_RZ // 4], dt.float16, tag="rzhh",
                                     bufs=8, name=f"rzhh_{idx}_{q}")
                    wdma(t_rzhh[:, :], base + F_RZ + q * F_RZ // 4, F_RZ // 4)
                    rzhh.append(t_rzhh)
                for h in range(2):
                    t_nnih = wp.tile([128, F_NN // 2], dt.float16, tag="nnih",
                                     bufs=4, name=f"nnih_{idx}_{h}")
                    wdma(t_nnih[:, :], base + 2 * F_RZ + h * F_NN // 2,
                         F_NN // 2)
                    nnih.append(t_nnih)
                    t_nnhh = wp.tile([128, F_NN // 2], dt.float16, tag="nnhh",
                                     bufs=4, name=f"nnhh_{idx}_{h}")
                    wdma(t_nnhh[:, :], base + 2 * F_RZ + F_NN + h * F_NN // 2,
                         F_NN // 2)
                    nnhh.append(t_nnhh)

                psum_rz = pp.tile([1, 2 * CH], dt.float32, tag="rz",
                                  name=f"psum_rz_{idx}")
                psum_nn = pp.tile([1, 2 * CH], dt.float32, tag="nn",
                                  name=f"psum_nn_{idx}")

                for k in range(KT):
                    hk, ko = divmod(k, 4)
                    nc.tensor.matmul(
                        out=psum_rz[:, :],
                        lhsT=xt_sb[:, k:k + 1],
                        rhs=rzih[hk][:, ko * 512:(ko + 1) * 512],
                        start=(k == 0), stop=False)
                    nc.tensor.matmul(
                        out=psum_rz[:, :],
                        lhsT=ht_sb[:, idx * KT + k:idx * KT + k + 1],
                        rhs=rzhh[hk][:, ko * 512:(ko + 1) * 512],
                        start=False, stop=False)
                nc.tensor.matmul(
                    out=psum_rz[:, :], lhsT=ones16[:, :],
                    rhs=bias_sb[:, idx * 1024:idx * 1024 + 512],
                    start=False, stop=True)

                # bias first: opens the accumulation group full-width so the
                # two lhsT-distinct halves can accumulate without their own
                # start=True (one PSUM group per bank region).
                nc.tensor.matmul(
                    out=psum_nn[:, :], lhsT=ones16[:, :],
                    rhs=bias_sb[:, idx * 1024 + 512:idx * 1024 + 1024],
                    start=True, stop=False)
                for k in range(KT):
                    hk, ko = divmod(k, 8)
                    nc.tensor.matmul(
                        out=psum_nn[:, 0:CH],
                        lhsT=xt_sb[:, k:k + 1],
                        rhs=nnih[hk][:, ko * CH:(ko + 1) * CH],
                        start=False, stop=False)
                    nc.tensor.matmul(
                        out=psum_nn[:, CH:2 * CH],
                        lhsT=ht_sb[:, idx * KT + k:idx * KT + k + 1],
                        rhs=nnhh[hk][:, ko * CH:(ko + 1) * CH],
                        start=False, stop=(k == KT - 1))

                # ---- elementwise GRU tail (partition 0) ----
                rz_sb = wk.tile([1, 2 * CH], dt.float32, tag="rz_sb",
                                name=f"rz_sb_{idx}")
                nc.scalar.activation(rz_sb[:, :], psum_rz[:, :],
                                     mybir.ActivationFunctionType.Sigmoid)
                t1 = wk.tile([1, CH], dt.float32, tag="t1", name=f"t1_{idx}")
                nc.vector.tensor_mul(out=t1[:, :], in0=rz_sb[:, 0:CH],
                                     in1=psum_nn[:, CH:2 * CH])
                t2 = wk.tile([1, CH], dt.float32, tag="t2", name=f"t2_{idx}")
                nc.vector.tensor_add(out=t2[:, :], in0=t1[:, :],
                                     in1=psum_nn[:, 0:CH])
                n_sb = wk.tile([1, CH], dt.float32, tag="n_sb",
                               name=f"n_sb_{idx}")
                nc.scalar.activation(n_sb[:, :], t2[:, :],
                                     mybir.ActivationFunctionType.Tanh)
                mem_row = aux_sb[:, AUX_MEM + i * CH:AUX_MEM + (i + 1) * CH]
                res_row = aux_sb[:, AUX_RES + i * CH:AUX_RES + (i + 1) * CH]
                d_sb = wk.tile([1, CH], dt.float32, tag="d_sb",
                               name=f"d_sb_{idx}")
                nc.vector.tensor_sub(out=d_sb[:, :], in0=mem_row, in1=n_sb[:, :])
                u_sb = wk.tile([1, CH], dt.float32, tag="u_sb",
                               name=f"u_sb_{idx}")
                nc.vector.tensor_mul(out=u_sb[:, :], in0=d_sb[:, :],
                                     in1=rz_sb[:, CH:2 * CH])
                h_new = staging[:, OUT_MEM + i * CH:OUT_MEM + (i + 1) * CH]
                nc.vector.tensor_add(out=h_new, in0=u_sb[:, :], in1=n_sb[:, :])
                s_sb = wk.tile([1, CH], dt.float32, tag="s_sb",
                               name=f"s_sb_{idx}")
                nc.vector.tensor_add(out=s_sb[:, :], in0=h_new, in1=res_row)
                res_out = staging[:, OUT_RES + i * CH:OUT_RES + (i + 1) * CH]
                nc.scalar.activation(res_out, s_sb[:, :],
                                     mybir.ActivationFunctionType.Sigmoid)

                tr_mm(h_new,
                      aux_sb[:, AUX_CV + D + idx:AUX_CV + D + idx + 1])
                tr_mm(res_out,
                      aux_sb[:, AUX_CV + 2 * D + idx:AUX_CV + 2 * D + idx + 1])

            # ---- output Linear: partial_out = out_w[:, cols].T-packed @ comp ----
            compT = sp.tile([128, 2], dt.float16)
            nc.vector.tensor_copy(out=compT[:, :], in_=psum_tr[:, :])

            for j in range(4):
                psum_po = pp.tile([1, 512], dt.float32, tag="po",
                                  name=f"psum_po_{j}")
                for t in range(2):
                    nc.tensor.matmul(
                        out=psum_po[:, :],
                        lhsT=compT[:, t:t + 1],
                        rhs=ow_sb[t][:, j * 512:(j + 1) * 512],
                        start=(t == 0), stop=(t == 1))
                nc.vector.tensor_copy(out=staging[:, j * 512:(j + 1) * 512],
                                      in_=psum_po[:, :])

            nc.sync.dma_start(out=outbuf[:, :], in_=staging[:, :])

    nc.compile()
    return nc


def _fingerprint(*arrs):
    out = []
    for a in arrs:
        f = np.asarray(a).reshape(-1)
        stride = max(1, f.size // 64)
        out.append((a.shape, float(f[::stride].astype(np.float64).sum())))
    return tuple(out)


def _pack_block(block_f16):
    """[..., R, K] fp16 -> [..., 128, KT*R] moving-operand layout."""
    shp = block_f16.shape
    R, K = shp[-2], shp[-1]
    kt = K // 128
    bt = np.swapaxes(block_f16, -1, -2)           # [..., K, R]
    bt = bt.reshape(*shp[:-2], kt, 128, R)
    bt = np.swapaxes(bt, -3, -2)                  # [..., 128, kt, R]
    return np.ascontiguousarray(bt).reshape(*shp[:-2], 128, kt * R)


def _pack_weights(w_ih, w_hh, out_w, active):
    na = len(active)
    wpack = np.empty((NCORES, 128, na * F_LAYER + F_OW), F16)

    act = list(active)
    # gates split [3, NCORES, CH] over the 3H dim
    wi = w_ih.reshape(D, 3, NCORES, CH, I)[act].astype(F16)  # [na,3,NC,CH,I]
    wh = w_hh.reshape(D, 3, NCORES, CH, I)[act].astype(F16)
    rz_i = np.concatenate([wi[:, 0], wi[:, 1]], axis=2)      # [na,NC,512,I]
    rz_h = np.concatenate([wh[:, 0], wh[:, 1]], axis=2)
    nn_i = wi[:, 2]                                          # [na,NC,256,I]
    nn_h = wh[:, 2]
    p_rz_i = _pack_block(rz_i)                               # [na,NC,128,F_RZ]
    p_rz_h = _pack_block(rz_h)
    p_nn_i = _pack_block(nn_i)                               # [na,NC,128,F_NN]
    p_nn_h = _pack_block(nn_h)
    for a in range(na):
        base = a * F_LAYER
        wpack[:, :, base:base + F_RZ] = p_rz_i[a]
        wpack[:, :, base + F_RZ:base + 2 * F_RZ] = p_rz_h[a]
        wpack[:, :, base + 2 * F_RZ:base + 2 * F_RZ + F_NN] = p_nn_i[a]
        wpack[:, :, base + 2 * F_RZ + F_NN:base + F_LAYER] = p_nn_h[a]

    # out_w column slices: pack[c, p, t*O + n] = out_w[n, c*CH + t*128 + p]
    owt = out_w.astype(F16).reshape(O, NCORES, 2, 128)
    owt = np.ascontiguousarray(owt.transpose(1, 3, 2, 0))    # [NC,128,2,O]
    wpack[:, :, na * F_LAYER:] = owt.reshape(NCORES, 128, F_OW)
    return wpack


def _prepare(inputs):
    step = int(np.asarray(inputs["step"]))
    active = _active_layers(step)
    na = len(active)

    x = np.asarray(inputs["x"], F32)
    memory = np.asarray(inputs["memory"], F32)
    residual = np.asarray(inputs["residual"], F32)
    b_ih = np.asarray(inputs["b_ih"], F32)
    b_hh = np.asarray(inputs["b_hh"], F32)

    key = (active, _fingerprint(inputs["w_ih"], inputs["w_hh"],
                                inputs["out_w"]))
    if key not in _PACK_CACHE:
        _PACK_CACHE.clear()
        _PACK_CACHE[key] = _pack_weights(
            np.asarray(inputs["w_ih"], F32), np.asarray(inputs["w_hh"], F32),
            np.asarray(inputs["out_w"], F32), active)
    wpack = _PACK_CACHE[key]

    xt = np.ascontiguousarray(x.astype(F16).reshape(KT, 128).T)
    ht = np.ascontiguousarray(
        memory[list(active)].astype(F16).reshape(na, KT, 128)
        .transpose(2, 0, 1)).reshape(128, na * KT)

    bsum = (b_ih + b_hh).reshape(D, 3, NCORES, CH)
    bi = b_ih.reshape(D, 3, NCORES, CH)
    bh = b_hh.reshape(D, 3, NCORES, CH)
    bias16 = np.empty((NCORES, 1, na * 1024), F16)
    for a, i in enumerate(active):
        bias16[:, 0, a * 1024:a * 1024 + 256] = bsum[i, 0].astype(F16)
        bias16[:, 0, a * 1024 + 256:a * 1024 + 512] = bsum[i, 1].astype(F16)
        bias16[:, 0, a * 1024 + 512:a * 1024 + 768] = bi[i, 2].astype(F16)
        bias16[:, 0, a * 1024 + 768:a * 1024 + 1024] = bh[i, 2].astype(F16)

    aux32 = np.zeros((NCORES, 1, AUX_LEN), F32)
    aux32[:, 0, AUX_MEM:AUX_MEM + D * CH] = (
        memory.reshape(D, NCORES, CH).transpose(1, 0, 2).reshape(NCORES, -1))
    aux32[:, 0, AUX_RES:AUX_RES + D * CH] = (
        residual.reshape(D, NCORES, CH).transpose(1, 0, 2).reshape(NCORES, -1))
    for a, i in enumerate(active):
        aux32[:, 0, AUX_CV + a] = 1.0 / (i + 1) ** 2          # fused weight
        aux32[:, 0, AUX_CV + D + a] = 1.0 / (a + 1) ** 2      # positional
        aux32[:, 0, AUX_CV + 2 * D + a] = 1.0 / (i + 1) ** 2  # residual
    inactive = [i for i in range(D) if i not in active]
    if inactive:
        base = np.zeros(H, F32)
        for i in inactive:
            base += residual[i] / (i + 1) ** 2
        aux32[:, 0, AUX_BASE:AUX_BASE + CH] = base.reshape(NCORES, CH)

    in_maps = [
        {"wpack": wpack[c], "xt": xt, "ht": ht,
         "bias16": bias16[c], "aux32": aux32[c]}
        for c in range(NCORES)
    ]
    return active, in_maps


def _assemble(inputs, active, per_core):
    memory = np.asarray(inputs["memory"], F32)
    residual = np.asarray(inputs["residual"], F32)
    out_b = np.asarray(inputs["out_b"], F32)

    stacked = np.stack([per_core[c][0] for c in range(NCORES)])  # [NC, OUT_LEN]
    output = stacked[:, OUT_P:OUT_P + O].sum(axis=0) + out_b
    new_mem = memory.copy()
    new_res = residual.copy()
    for c in range(NCORES):
        sl = slice(c * CH, (c + 1) * CH)
        for i in active:
            new_mem[i, sl] = stacked[c, OUT_MEM + i * CH:OUT_MEM + (i + 1) * CH]
            new_res[i, sl] = stacked[c, OUT_RES + i * CH:OUT_RES + (i + 1) * CH]
    return output, new_res, new_mem


def _execute(inputs, trace=False, **kwargs):
    active, in_maps = _prepare(inputs)
    if active not in _KERNEL_CACHE:
        _KERNEL_CACHE[active] = _build_nc(active)
    nc = _KERNEL_CACHE[active]
    try:
        res = run_bass_kernel_spmd(nc, in_maps, list(range(NCORES)),
                                   trace=trace, **kwargs)
    except Exception:
        # The first execution of a freshly compiled NEFF under the NTFF
        # profiler is flaky (NRT_EXEC_UNIT_UNRECOVERABLE); one retry after
        # the warm load consistently succeeds.
        import time as _time
        _time.sleep(2.0)
        res = run_bass_kernel_spmd(nc, in_maps, list(range(NCORES)),
                                   trace=trace, **kwargs)
    per_core = [res.results[c]["outbuf"] for c in range(NCORES)]
    return _assemble(inputs, active, per_core), res


def kernel(**inputs):
    outs, _ = _execute(inputs)
    return outs


def kernel_profiled(inputs, warmup=True, **kwargs):
    if warmup:
        _execute(inputs, trace=False)
    outs, res = _execute(inputs, trace=True, **kwargs)
    return outs, res
